# revision 4
# baseline (speedup 1.0000x reference)
"""Trainium2 Bass kernel for CIN layer:
    out[b,c,d] = sum_{h,m} W[c, h*M+m] * xk[b,h,d] * x0[b,m,d] + bias[c]

Shapes (hardcoded): x0 [512,40,64] f32, xk [512,128,64] f32,
W [128,5120] f32, b [128] f32 -> out [512,128,64] f32.

Strategy: data-parallel over batch B across 8 cores (64 batches/core).
Per core, columns are the 64*64=4096 (b,d) pairs. The 5120-long
contraction is ordered m-major: chunk m holds rows (m, h=0..127).
Per m:
  outer_m[h, col] = xk[h, col] * x0bc_m[col]    (DVE tensor_tensor, bf16)
  psum[g] += W_mT[h,c].T @ outer_m[:, g*512:...] (PE, accumulate over m)
x0bc_m is x0's row m replicated across the 128 partitions; the
replication is produced host-side (pure layout, no arithmetic) and
streamed from HBM by DMA. W is host-pre-transposed to [h, m, c].
Bias-add is fused into the PSUM->SBUF eviction on ScalarE.
"""

import numpy as np
import ml_dtypes

B, M, H, D, C = 512, 40, 128, 64, 128
N_CORES = 8
BC = B // N_CORES          # 64 batches per core
COLS = BC * D              # 4096 (b,d) columns per core
NG = 8                     # PSUM groups
GW = COLS // NG            # 512 columns per group

_cache = {}


def _build(reps=1):
    import contextlib

    import concourse.bacc as bacc
    import concourse.mybir as mybir
    from concourse.tile import TileContext

    f32 = mybir.dt.float32
    bf16 = mybir.dt.bfloat16

    nc = bacc.Bacc("TRN2", debug=False, num_devices=N_CORES)

    xk_d = nc.dram_tensor("xk_in", [BC, H, D], f32, kind="ExternalInput")
    x0r_d = nc.dram_tensor("x0rep_in", [M, 128, COLS], bf16, kind="ExternalInput")
    wT_d = nc.dram_tensor("wT_in", [H, M, C], f32, kind="ExternalInput")
    bias_d = nc.dram_tensor("bias_in", [C, 1], f32, kind="ExternalInput")
    out_d = nc.dram_tensor("out", [BC, C, D], f32, kind="ExternalOutput")

    with TileContext(nc) as tc:
        with (
            tc.tile_pool(name="const", bufs=1) as cpool,
            tc.tile_pool(name="work", bufs=3) as wpool,
            tc.tile_pool(name="outp", bufs=2) as opool,
            tc.tile_pool(name="psum", bufs=1, space="PSUM") as ppool,
        ):
            # ---- load + cast constants ----
            xk_f32 = cpool.tile([128, COLS], f32)
            nc.sync.dma_start(out=xk_f32, in_=xk_d.ap().rearrange("b h d -> h b d"))
            xk_sb = cpool.tile([128, COLS], bf16)
            nc.vector.tensor_copy(xk_sb, xk_f32)

            wT_f32 = cpool.tile([128, M * C], f32)
            nc.sync.dma_start(out=wT_f32, in_=wT_d.ap().rearrange("h m c -> h (m c)"))
            w_sb = cpool.tile([128, M * C], bf16)
            nc.vector.tensor_copy(w_sb, wT_f32)

            bias_sb = cpool.tile([128, 1], f32)
            nc.sync.dma_start(out=bias_sb, in_=bias_d.ap())

            loop_ctx = tc.For_i(0, reps, 1) if reps > 1 else contextlib.nullcontext()
            with loop_ctx:
                psums = []
                for g in range(NG):
                    ps = ppool.tile([128, GW], f32, name=f"ps{g}", tag=f"ps{g}")
                    psums.append(ps)

                # ---- main loop over the 40 m-chunks ----
                for m in range(M):
                    x0bc = wpool.tile([128, COLS], bf16, name=f"x0bc{m}", tag="x0bc")
                    nc.sync.dma_start(out=x0bc, in_=x0r_d.ap()[m])
                    outer = wpool.tile(
                        [128, COLS], bf16, name=f"outer{m}", tag="outer"
                    )
                    nc.vector.tensor_mul(outer, xk_sb, x0bc)
                    for g in range(NG):
                        nc.tensor.matmul(
                            psums[g],
                            lhsT=w_sb[:, m * C:(m + 1) * C],
                            rhs=outer[:, g * GW:(g + 1) * GW],
                            start=(m == 0),
                            stop=(m == M - 1),
                        )

                # ---- bias add + store ----
                out_ap = out_d.ap().rearrange("b c d -> c b d")
                bpg = BC // NG  # batches per group
                for g in range(NG):
                    out_sb = opool.tile([128, GW], f32, name=f"osb{g}", tag="osb")
                    nc.scalar.activation(
                        out_sb,
                        psums[g],
                        mybir.ActivationFunctionType.Identity,
                        bias=bias_sb[:, 0:1],
                        scale=1.0,
                    )
                    nc.sync.dma_start(
                        out=out_ap[:, g * bpg:(g + 1) * bpg, :], in_=out_sb
                    )

    nc.compile()
    return nc


def _prep_host(x0, xk, W, b):
    """Host-side layout prep (no arithmetic): shard, transpose, replicate."""
    wT = np.ascontiguousarray(
        W.reshape(C, H, M).transpose(1, 2, 0)
    )  # [h, m, c] f32
    bias = np.ascontiguousarray(b.reshape(C, 1)).astype(np.float32)
    in_maps = []
    for k in range(N_CORES):
        x0s = x0[k * BC:(k + 1) * BC]            # [BC, M, D]
        xks = np.ascontiguousarray(xk[k * BC:(k + 1) * BC])  # [BC, H, D]
        x0rows = np.ascontiguousarray(x0s.transpose(1, 0, 2)).reshape(M, COLS)
        x0rep = np.ascontiguousarray(
            np.broadcast_to(
                x0rows.astype(ml_dtypes.bfloat16)[:, None, :], (M, 128, COLS)
            )
        )
        in_maps.append(
            {
                "xk_in": xks.astype(np.float32, copy=False),
                "x0rep_in": x0rep,
                "wT_in": wT,
                "bias_in": bias,
            }
        )
    return in_maps


def _run(in_maps, **kwargs):
    from concourse import bass_utils

    if "nc" not in _cache:
        _cache["nc"] = _build()
    return bass_utils.run_bass_kernel_spmd(
        _cache["nc"], in_maps, core_ids=list(range(N_CORES)), **kwargs
    )


def kernel(x0, xk, W, b, _bench=[None]):
    x0 = np.asarray(x0, dtype=np.float32)
    xk = np.asarray(xk, dtype=np.float32)
    W = np.asarray(W, dtype=np.float32)
    b = np.asarray(b, dtype=np.float32)
    in_maps = _prep_host(x0, xk, W, b)
    res = _run(in_maps)
    _bench[0] = res
    out = np.concatenate([r["out"] for r in res.results], axis=0)
    return out.astype(np.float32, copy=False)


# revision 8
# speedup vs baseline: 1.2753x; 1.2753x over previous
"""Trainium2 Bass kernel for CIN layer:
    out[b,c,d] = sum_{h,m} W[c, h*M+m] * xk[b,h,d] * x0[b,m,d] + bias[c]

Shapes (hardcoded): x0 [512,40,64] f32, xk [512,128,64] f32,
W [128,5120] f32, b [128] f32 -> out [512,128,64] f32.

Strategy: data-parallel over batch B across 8 cores (64 batches/core).
Per core, columns are the 64*64=4096 (b,d) pairs. The 5120-long (h,m)
contraction is split into 40 chunks of 128 rows with a mixed-radix
partition layout: chunk (g, j) covers m in the 8-wide group g (5 groups)
x h in the 16-wide block j (8 blocks); partition p holds
(m = 8g + p//16, h = 16j + p%16). Then
  outer[p, col] = xkrep_j[p, col] * x0bc_g[p, col]  (DVE TT, bf16 2x)
  psum[q] += w3[g,j][p,c].T @ outer[:, q*512:...]   (PE, accum 40 chunks)
where xkrep_j (xk h-block replicated 8x along partitions) and x0bc_g
(x0 m-group rows replicated 16x) are produced host-side (pure layout,
no arithmetic): only 8 + 5 = 13 replicated tiles total, each reused
across the other loop axis - 3.2x less DMA than a full x0 broadcast.
W is host-gathered to match the chunk layout. Bias-add is fused into
the PSUM->SBUF eviction on ScalarE.
"""

import numpy as np
import ml_dtypes

B, M, H, D, C = 512, 40, 128, 64, 128
N_CORES = 8
BC = B // N_CORES          # 64 batches per core
COLS = BC * D              # 4096 (b,d) columns per core
NG = 8                     # PSUM groups
GW = COLS // NG            # 512 columns per group
MG = 8                     # m-values per chunk group
NMG = M // MG              # 5 m-groups
HB = 128 // MG             # 16 h-values per block
NHB = H // HB              # 8 h-blocks
NCHUNK = NMG * NHB         # 40 contraction chunks

_cache = {}


def _build(reps=1):
    import contextlib

    import concourse.bacc as bacc
    import concourse.mybir as mybir
    from concourse.tile import TileContext

    f32 = mybir.dt.float32
    bf16 = mybir.dt.bfloat16

    nc = bacc.Bacc("TRN2", debug=False, num_devices=N_CORES)

    xkr_d = nc.dram_tensor("xkrep_in", [NHB, 128, COLS], bf16, kind="ExternalInput")
    x0b_d = nc.dram_tensor("x0bc_in", [NMG, 128, COLS], bf16, kind="ExternalInput")
    w3_d = nc.dram_tensor("w3_in", [NCHUNK, 128, C], f32, kind="ExternalInput")
    bias_d = nc.dram_tensor("bias_in", [C, 1], f32, kind="ExternalInput")
    out_d = nc.dram_tensor("out", [BC, C, D], f32, kind="ExternalOutput")

    with TileContext(nc) as tc:
        with (
            tc.tile_pool(name="const", bufs=1) as cpool,
            tc.tile_pool(name="work", bufs=4) as wpool,
            tc.tile_pool(name="outp", bufs=2) as opool,
            tc.tile_pool(name="psum", bufs=1, space="PSUM") as ppool,
        ):
            # ---- load constants / replicated operand tiles ----
            w3_f32 = cpool.tile([128, NCHUNK * C], f32)
            nc.sync.dma_start(
                out=w3_f32, in_=w3_d.ap().rearrange("k p c -> p k c")
            )
            w3_sb = cpool.tile([128, NCHUNK * C], bf16)
            # cast on ScalarE: keeps VectorE free for the main-loop products
            nc.scalar.copy(w3_sb, w3_f32)

            bias_sb = cpool.tile([128, 1], f32)
            nc.sync.dma_start(out=bias_sb, in_=bias_d.ap())

            xkreps = []
            for j in range(NHB):
                xkr = cpool.tile([128, COLS], bf16, name=f"xkr{j}", tag=f"xkr{j}")
                nc.sync.dma_start(out=xkr, in_=xkr_d.ap()[j])
                xkreps.append(xkr)
            x0bcs = []
            for g in range(NMG):
                x0b = cpool.tile([128, COLS], bf16, name=f"x0b{g}", tag=f"x0b{g}")
                nc.sync.dma_start(out=x0b, in_=x0b_d.ap()[g])
                x0bcs.append(x0b)

            loop_ctx = tc.For_i(0, reps, 1) if reps > 1 else contextlib.nullcontext()
            with loop_ctx:
                psums = []
                for q in range(NG):
                    ps = ppool.tile([128, GW], f32, name=f"ps{q}", tag=f"ps{q}")
                    psums.append(ps)

                # ---- main loop over the 40 contraction chunks ----
                for k in range(NCHUNK):
                    g, j = divmod(k, NHB)
                    outer = wpool.tile(
                        [128, COLS], bf16, name=f"outer{k}", tag="outer"
                    )
                    nc.vector.tensor_mul(outer, xkreps[j], x0bcs[g])
                    for q in range(NG):
                        nc.tensor.matmul(
                            psums[q],
                            lhsT=w3_sb[:, k * C:(k + 1) * C],
                            rhs=outer[:, q * GW:(q + 1) * GW],
                            start=(k == 0),
                            stop=(k == NCHUNK - 1),
                        )

                # ---- bias add + store ----
                out_ap = out_d.ap().rearrange("b c d -> c b d")
                bpg = BC // NG  # batches per group
                for q in range(NG):
                    out_sb = opool.tile([128, GW], f32, name=f"osb{q}", tag="osb")
                    nc.scalar.activation(
                        out_sb,
                        psums[q],
                        mybir.ActivationFunctionType.Identity,
                        bias=bias_sb[:, 0:1],
                        scale=1.0,
                    )
                    nc.sync.dma_start(
                        out=out_ap[:, q * bpg:(q + 1) * bpg, :], in_=out_sb
                    )

    nc.compile()
    return nc


def _prep_host(x0, xk, W, b):
    """Host-side layout prep (no arithmetic): shard, transpose, replicate."""
    part = np.arange(128)
    hh = (part % HB)[None, :] + HB * np.arange(NHB)[:, None]   # [NHB, 128]
    mm = (part // HB)[None, :] + MG * np.arange(NMG)[:, None]  # [NMG, 128]

    Wr = W.reshape(C, H, M)
    w3 = np.empty((NCHUNK, 128, C), np.float32)
    for g in range(NMG):
        for j in range(NHB):
            w3[g * NHB + j] = Wr[:, hh[j], mm[g]].T
    bias = np.ascontiguousarray(b.reshape(C, 1)).astype(np.float32)

    in_maps = []
    for k in range(N_CORES):
        x0s = x0[k * BC:(k + 1) * BC]            # [BC, M, D]
        xks = xk[k * BC:(k + 1) * BC]            # [BC, H, D]
        xk2 = (
            np.ascontiguousarray(xks.transpose(1, 0, 2))
            .reshape(H, COLS)
            .astype(ml_dtypes.bfloat16)
        )
        x02 = (
            np.ascontiguousarray(x0s.transpose(1, 0, 2))
            .reshape(M, COLS)
            .astype(ml_dtypes.bfloat16)
        )
        in_maps.append(
            {
                "xkrep_in": np.ascontiguousarray(xk2[hh]),
                "x0bc_in": np.ascontiguousarray(x02[mm]),
                "w3_in": w3,
                "bias_in": bias,
            }
        )
    return in_maps


def _run(in_maps, **kwargs):
    from concourse import bass_utils

    if "nc" not in _cache:
        _cache["nc"] = _build()
    return bass_utils.run_bass_kernel_spmd(
        _cache["nc"], in_maps, core_ids=list(range(N_CORES)), **kwargs
    )


def kernel(x0, xk, W, b, _bench=[None]):
    x0 = np.asarray(x0, dtype=np.float32)
    xk = np.asarray(xk, dtype=np.float32)
    W = np.asarray(W, dtype=np.float32)
    b = np.asarray(b, dtype=np.float32)
    in_maps = _prep_host(x0, xk, W, b)
    res = _run(in_maps)
    _bench[0] = res
    out = np.concatenate([r["out"] for r in res.results], axis=0)
    return out.astype(np.float32, copy=False)


# revision 13
# speedup vs baseline: 1.5649x; 1.2271x over previous
"""Trainium2 Bass kernel for CIN layer:
    out[b,c,d] = sum_{h,m} W[c, h*M+m] * xk[b,h,d] * x0[b,m,d] + bias[c]

Shapes (hardcoded): x0 [512,40,64] f32, xk [512,128,64] f32,
W [128,5120] f32, b [128] f32 -> out [512,128,64] f32.

Strategy: data-parallel over batch B across 8 cores (64 batches/core).
Per core, columns are the 64*64=4096 (b,d) pairs. The 5120-long (h,m)
contraction is split into 40 chunks of 128 rows with a mixed-radix
partition layout: chunk (g, j) covers m in the 8-wide group g (5 groups)
x h in the 16-wide block j (8 blocks); partition p holds
(m = 8g + p//16, h = 16j + p%16). Then
  outer[p, col] = xkrep_j[p, col] * x0bc_g[p, col]  (DVE TT, bf16 2x)
  psum[q] += w3[g,j][p,c].T @ outer[:, q*512:...]   (PE, accum 40 chunks)
where xkrep_j (xk h-block replicated 8x along partitions) and x0bc_g
(x0 m-group rows replicated 16x) are produced host-side (pure layout,
no arithmetic): only 8 + 5 = 13 replicated tiles total, each reused
across the other loop axis - 3.2x less DMA than a full x0 broadcast.
W is host-gathered to match the chunk layout. Bias-add is fused into
the PSUM->SBUF eviction on ScalarE.
"""

import numpy as np
import ml_dtypes

B, M, H, D, C = 512, 40, 128, 64, 128
N_CORES = 8
BC = B // N_CORES          # 64 batches per core
COLS = BC * D              # 4096 (b,d) columns per core
NG = 8                     # PSUM groups
GW = COLS // NG            # 512 columns per group
MG = 8                     # m-values per chunk group
NMG = M // MG              # 5 m-groups
HB = 128 // MG             # 16 h-values per block
NHB = H // HB              # 8 h-blocks
NCHUNK = NMG * NHB         # 40 contraction chunks

_cache = {}


def _build(reps=1):
    import contextlib

    import concourse.bacc as bacc
    import concourse.mybir as mybir
    from concourse.tile import TileContext

    f32 = mybir.dt.float32
    bf16 = mybir.dt.bfloat16

    nc = bacc.Bacc("TRN2", debug=False, num_devices=N_CORES)

    xkr_d = nc.dram_tensor("xkrep_in", [NHB, 128, COLS], bf16, kind="ExternalInput")
    x0b_d = nc.dram_tensor("x0bc_in", [NMG, 128, COLS], bf16, kind="ExternalInput")
    w3_d = nc.dram_tensor("w3_in", [NCHUNK, 128, C], bf16, kind="ExternalInput")
    bias_d = nc.dram_tensor("bias_in", [C, 1], f32, kind="ExternalInput")
    out_d = nc.dram_tensor("out", [BC, C, D], f32, kind="ExternalOutput")

    with TileContext(nc) as tc:
        with (
            tc.tile_pool(name="const", bufs=1) as cpool,
            tc.tile_pool(name="work", bufs=6) as wpool,
            tc.tile_pool(name="outp", bufs=2) as opool,
            tc.tile_pool(name="psum", bufs=1, space="PSUM") as ppool,
        ):
            # ---- load constants / replicated operand tiles ----
            w3_sb = cpool.tile([128, NCHUNK * C], bf16)
            nc.sync.dma_start(
                out=w3_sb, in_=w3_d.ap().rearrange("k p c -> p k c")
            )

            bias_sb = cpool.tile([128, 1], f32)
            nc.sync.dma_start(out=bias_sb, in_=bias_d.ap())

            # interleave the replicated-tile loads roughly in first-use
            # order (chunk loop is j-fastest within g=0 first)
            xkreps = [None] * NHB
            x0bcs = [None] * NMG
            load_order = [("x", 0), ("0", 0), ("x", 1), ("x", 2), ("0", 1),
                          ("x", 3), ("x", 4), ("0", 2), ("x", 5), ("x", 6),
                          ("0", 3), ("x", 7), ("0", 4)]
            for kind, i in load_order:
                if kind == "x":
                    xkr = cpool.tile(
                        [128, COLS], bf16, name=f"xkr{i}", tag=f"xkr{i}"
                    )
                    nc.sync.dma_start(out=xkr, in_=xkr_d.ap()[i])
                    xkreps[i] = xkr
                else:
                    x0b = cpool.tile(
                        [128, COLS], bf16, name=f"x0b{i}", tag=f"x0b{i}"
                    )
                    nc.sync.dma_start(out=x0b, in_=x0b_d.ap()[i])
                    x0bcs[i] = x0b

            loop_ctx = (
                tc.For_i(0, reps, 1, hint_engines=(mybir.EngineType.PE,))
                if reps > 1
                else contextlib.nullcontext()
            )
            with loop_ctx:
                psums = []
                for q in range(NG):
                    ps = ppool.tile([128, GW], f32, name=f"ps{q}", tag=f"ps{q}")
                    psums.append(ps)

                # ---- main loop over the 40 contraction chunks ----
                for k in range(NCHUNK):
                    g, j = divmod(k, NHB)
                    outer = wpool.tile(
                        [128, COLS], bf16, name=f"outer{k}", tag="outer"
                    )
                    nc.vector.tensor_mul(outer, xkreps[j], x0bcs[g])
                    for q in range(NG):
                        nc.tensor.matmul(
                            psums[q],
                            lhsT=w3_sb[:, k * C:(k + 1) * C],
                            rhs=outer[:, q * GW:(q + 1) * GW],
                            start=(k == 0),
                            stop=(k == NCHUNK - 1),
                        )

                # ---- bias add + store ----
                out_ap = out_d.ap().rearrange("b c d -> c b d")
                bpg = BC // NG  # batches per group
                for q in range(NG):
                    out_sb = opool.tile([128, GW], f32, name=f"osb{q}", tag="osb")
                    nc.scalar.activation(
                        out_sb,
                        psums[q],
                        mybir.ActivationFunctionType.Identity,
                        bias=bias_sb[:, 0:1],
                        scale=1.0,
                    )
                    nc.sync.dma_start(
                        out=out_ap[:, q * bpg:(q + 1) * bpg, :], in_=out_sb
                    )

    nc.compile()
    return nc


def _prep_host(x0, xk, W, b):
    """Host-side layout prep (no arithmetic): shard, transpose, replicate."""
    part = np.arange(128)
    hh = (part % HB)[None, :] + HB * np.arange(NHB)[:, None]   # [NHB, 128]
    mm = (part // HB)[None, :] + MG * np.arange(NMG)[:, None]  # [NMG, 128]

    Wr = W.reshape(C, H, M)
    w3 = np.empty((NCHUNK, 128, C), ml_dtypes.bfloat16)
    for g in range(NMG):
        for j in range(NHB):
            w3[g * NHB + j] = Wr[:, hh[j], mm[g]].T.astype(ml_dtypes.bfloat16)
    bias = np.ascontiguousarray(b.reshape(C, 1)).astype(np.float32)

    in_maps = []
    for k in range(N_CORES):
        x0s = x0[k * BC:(k + 1) * BC]            # [BC, M, D]
        xks = xk[k * BC:(k + 1) * BC]            # [BC, H, D]
        xk2 = (
            np.ascontiguousarray(xks.transpose(1, 0, 2))
            .reshape(H, COLS)
            .astype(ml_dtypes.bfloat16)
        )
        x02 = (
            np.ascontiguousarray(x0s.transpose(1, 0, 2))
            .reshape(M, COLS)
            .astype(ml_dtypes.bfloat16)
        )
        in_maps.append(
            {
                "xkrep_in": np.ascontiguousarray(xk2[hh]),
                "x0bc_in": np.ascontiguousarray(x02[mm]),
                "w3_in": w3,
                "bias_in": bias,
            }
        )
    return in_maps


def _run(in_maps, **kwargs):
    from concourse import bass_utils

    if "nc" not in _cache:
        _cache["nc"] = _build()
    return bass_utils.run_bass_kernel_spmd(
        _cache["nc"], in_maps, core_ids=list(range(N_CORES)), **kwargs
    )


def kernel(x0, xk, W, b, _bench=[None]):
    x0 = np.asarray(x0, dtype=np.float32)
    xk = np.asarray(xk, dtype=np.float32)
    W = np.asarray(W, dtype=np.float32)
    b = np.asarray(b, dtype=np.float32)
    in_maps = _prep_host(x0, xk, W, b)
    res = _run(in_maps)
    _bench[0] = res
    out = np.concatenate([r["out"] for r in res.results], axis=0)
    return out.astype(np.float32, copy=False)
